# revision 17
# baseline (speedup 1.0000x reference)
"""Trainium2 Bass kernel for nn_MultiHeadAttention_39883066311260.

Sparse multi-head attention with relative-position-bucket bias and a
double softmax (row-softmax * column-softmax), sharded one head per
NeuronCore across 8 cores (tensor parallel on the head dim).

Math (per head h, all on device):
    xq = LN(x_q); xk = LN(x_k)            (LN folded into the projections)
    q = xq Wq_h^T, k = xk Wk_h^T, v = xk Wv_h^T
    S = (q k^T + qk_rel[q, gpm]) / 8, masked -> -1e9-ish
    attn = softmax_row(S) * softmax_col(S) = T^2 * rinv[q] * cinv[k]
        with T = exp(S) (unstabilized; logits are O(10), masked -> exp=0)
    out_h = attn @ v;  gated; AllGather heads; out = G Wo^T + bo

The per-element bucket gather qk_rel[q, gpm[q,k]] runs as two custom DVE
ops (per-partition LUT via sum-of-exclusive-products over an int8-coded
index in {-2..2}; mask folded into the index on the host as code 64,
which matches no bucket and nets a -SHIFT logit via the exp bias).
"""

import numpy as np

import concourse.bass as bass
import concourse.bacc as bacc
import concourse.mybir as mybir
import concourse.tile as tile

H, DM, DK, PK = 8, 512, 64, 5
N_FULL = 2048
FP32 = mybir.dt.float32
F32R = mybir.dt.float32r
BF16 = mybir.dt.bfloat16
I8 = mybir.dt.int8
AF = mybir.ActivationFunctionType
ALU = mybir.AluOpType
SHIFT = 2048.0
EPS = 1e-6

# --------------------------------------------------------------------------
# Custom DVE ops: 6-entry per-partition LUT in two passes + fused combine.
# --------------------------------------------------------------------------
_DVE_OPS = {}


def _register_dve_ops():
    global _DVE_OPS
    if _DVE_OPS:
        return _DVE_OPS
    import concourse.dve_ops as D
    from concourse.dve_spec import (
        C0, C1, C2, C3, One, Spec, Src0, Src1, Zero, eq, sq, lower,
        _spill_c3_to_src1, _has_src1,
    )
    from concourse.dve_uop import DveOpSpec

    if any(op.name == "MHA_LUT_A" for op in D.OPS):
        _DVE_OPS = {op.name: op for op in D.OPS if op.name.startswith("MHA_")}
        return _DVE_OPS

    TWO = One + One
    NEG1 = Zero - One
    NEG2 = NEG1 + NEG1

    def build(name, spec):
        row = D._CUSTOM_DVE_ROW_BASE + len(D.OPS)
        D._SUB_OPCODE_FOR_NAME[name] = row
        shas = {}
        for ver in ("v3", "v4"):
            s = DveOpSpec(name=name, opcode=row, uops=lower(spec, ver=ver),
                          rd1_en=_has_src1(spec))
            shas[ver] = s.sha(ver)
        op = D.DveOp(name, spec, subdim=False, uops_sha=shas)
        D.OPS.append(op)
        D.CUSTOM_DVE_SPECS[name] = spec
        _DVE_OPS[name] = op
        return op

    # gidx is int8-encoded: bucket b -> b-2 in {-2..2}, masked -> 64.
    # Table values arrive pre-shifted by +SHIFT; the exp pass bias subtracts
    # SHIFT, so an unmatched index (masked) nets -SHIFT = a huge negative
    # logit whose exp underflows to 0, with no explicit mask compare.
    # pass A: encoded {0,1,2} = buckets {2,3,4} via s0/s1/in1(C3-spill).
    body_a = (eq(Src0, Zero) * C0 + eq(Src0, One) * C1
              + eq(Src0, TWO) * C3)
    build("MHA_LUT_A", Spec(
        body=_spill_c3_to_src1(body_a),
        reference=lambda in0, in1, s0, s1, imm2: (
            (in0 == 0) * s0 + (in0 == 1) * s1
            + (in0 == 2) * in1).astype(np.float32),
    ))
    # pass B: encoded {-1,-2} = buckets {1,0} added onto pass A's output.
    body_b = Src1 + eq(Src0, NEG1) * C0 + eq(Src0, NEG2) * C1
    build("MHA_LUT_B", Spec(
        body=body_b,
        reference=lambda in0, in1, s0, s1, imm2: (
            in1 + (in0 == -1) * s0 + (in0 == -2) * s1).astype(np.float32),
    ))
    # combine: attn = T^2 * rinv[q] * cinvB
    build("MHA_COMBINE", Spec(
        body=sq(Src0) * C0 * Src1,
        reference=lambda in0, in1, s0, s1, imm2: (
            in0.astype(np.float32) ** 2 * s0 * in1).astype(np.float32),
    ))
    return _DVE_OPS


# --------------------------------------------------------------------------
# The per-core Bass program (SPMD: identical program, per-core input values).
# --------------------------------------------------------------------------
_NC_CACHE = {}


def build_nc(n=N_FULL):
    if n in _NC_CACHE:
        return _NC_CACHE[n]
    ops = _register_dve_ops()
    LUT_A, LUT_B, COMBINE = ops["MHA_LUT_A"], ops["MHA_LUT_B"], ops["MHA_COMBINE"]
    from concourse.masks import make_identity

    nt = n // 128          # q/k tiles of 128
    nb = n // 512          # 512-wide PSUM bank chunks
    ncc = DM // 128        # channel chunks

    nc = bacc.Bacc("TRN2", num_devices=H)

    # ---- I/O ----
    xq_d = nc.dram_tensor("x_q", [n, DM], FP32, kind="ExternalInput")
    xk_d = nc.dram_tensor("x_k", [n, DM], FP32, kind="ExternalInput")
    gidx_d = nc.dram_tensor("gidx", [n, n], I8, kind="ExternalInput")
    wqT_d = nc.dram_tensor("wqT", [DM, DK], FP32, kind="ExternalInput")
    wkT_d = nc.dram_tensor("wkT", [DM, DK], FP32, kind="ExternalInput")
    wvT_d = nc.dram_tensor("wvT", [DM, DK], FP32, kind="ExternalInput")
    wgT_d = nc.dram_tensor("wgT", [DM, DK], FP32, kind="ExternalInput")
    woT_d = nc.dram_tensor("woT", [DM, DM], FP32, kind="ExternalInput")
    relT_d = nc.dram_tensor("rel_kT", [DK, PK], FP32, kind="ExternalInput")
    lnw_d = nc.dram_tensor("ln_w", [DM], FP32, kind="ExternalInput")
    lnb_d = nc.dram_tensor("ln_b", [DM], FP32, kind="ExternalInput")
    bg_d = nc.dram_tensor("bg_h", [DK, 1], FP32, kind="ExternalInput")
    bo_d = nc.dram_tensor("bo", [1, DM], FP32, kind="ExternalInput")

    attn_d = nc.dram_tensor("attn", [n, n], FP32, kind="ExternalOutput")
    out_d = nc.dram_tensor("out", [n, DM], FP32, kind="ExternalOutput")

    cc_in = nc.dram_tensor("cc_in", [DK, n], FP32)
    cc_out = nc.dram_tensor("cc_out", [DM, n], FP32, addr_space="Shared")

    with tile.TileContext(nc) as tc:
        with (
            tc.tile_pool(name="const", bufs=1) as const,
            tc.tile_pool(name="small", bufs=1) as small,
            tc.tile_pool(name="tslab", bufs=1) as tslab,
        ):
            # ---------- constants ----------
            ident = const.tile([128, 128], FP32)
            make_identity(nc, ident)
            identb = const.tile([128, 128], BF16)
            nc.vector.tensor_copy(identb, ident)
            ones_col = const.tile([128, 1], FP32)
            nc.vector.memset(ones_col, 1.0)
            ones_colb = const.tile([128, 128], BF16)
            nc.vector.memset(ones_colb, 1.0)
            ones_row = const.tile([1, 128], FP32)
            nc.vector.memset(ones_row, 1.0)
            ones_colr = const.tile([128, 1], F32R)
            nc.vector.tensor_copy(ones_colr, ones_col)
            identr = const.tile([128, 128], F32R)
            nc.vector.tensor_copy(identr, ident)
            ones_rowb = const.tile([1, 128], BF16)
            nc.vector.tensor_copy(ones_rowb, ones_row)
            eps_t = const.tile([1, 1], FP32)
            nc.vector.memset(eps_t, EPS)

            relT = const.tile([DK, PK], FP32)
            nc.sync.dma_start(out=relT, in_=relT_d[:, :])
            bg_t = const.tile([DK, 1], FP32)
            nc.sync.dma_start(out=bg_t, in_=bg_d[:, :])
            bo_t = const.tile([1, DM], FP32)
            nc.sync.dma_start(out=bo_t, in_=bo_d[:, :])
            # ln_w / ln_b in per-partition chunk form [128, ncc]
            lnw_pp = const.tile([128, ncc], FP32)
            nc.sync.dma_start(
                out=lnw_pp, in_=lnw_d.rearrange("(cc p) -> p cc", p=128))
            lnb_pp = const.tile([128, ncc], FP32)
            nc.sync.dma_start(
                out=lnb_pp, in_=lnb_d.rearrange("(cc p) -> p cc", p=128))

            wT = {}
            for name, dram in (("q", wqT_d), ("k", wkT_d), ("v", wvT_d),
                               ("g", wgT_d)):
                t = const.tile([128, ncc, DK], FP32, tag=f"w{name}")
                nc.sync.dma_start(
                    out=t, in_=dram.rearrange("(cc p) d -> p cc d", p=128))
                wT[name] = t
            woT = const.tile([128, ncc, DM], FP32)
            nc.sync.dma_start(
                out=woT, in_=woT_d.rearrange("(cc p) d -> p cc d", p=128))

            # ================= phase 0: LN + projections =================
            with (
                tc.tile_pool(name="xload", bufs=3) as xload,
                tc.tile_pool(name="xt", bufs=1) as xtp,
                tc.tile_pool(name="sqbuf", bufs=2) as sqp,
                tc.tile_pool(name="p0tmp", bufs=1) as p0tmp,
                tc.tile_pool(name="ps_row", bufs=1, space="PSUM") as ps_row,
                tc.tile_pool(name="ps0", bufs=2, space="PSUM") as ps0,
            ):
                rows_t = small.tile([9, n], FP32, tag="rows")
                R_MU = {"q": 0, "k": 3}
                R_IRS = {"q": 1, "k": 4}
                R_RSTD = {"q": 2, "k": 5}
                # ---------- load x and transpose to [DM, n] ----------
                xT = {}
                for key, dram in (("q", xq_d), ("k", xk_d)):
                    tiles = [xtp.tile([128, n], F32R, tag=f"xt{key}{cc}",
                                      name=f"xt{key}{cc}")
                             for cc in range(ncc)]
                    xT[key] = tiles
                    for i in range(nt):
                        xt_in = xload.tile([128, DM], FP32, tag="xin")
                        nc.sync.dma_start(
                            out=xt_in, in_=dram[i * 128:(i + 1) * 128, :])
                        ps = ps0.tile([128, 512], FP32, tag="ps0")
                        for cc in range(ncc):
                            nc.tensor.transpose(
                                ps[:, cc * 128:(cc + 1) * 128],
                                xt_in[:, cc * 128:(cc + 1) * 128], ident)
                        for cc in range(ncc):
                            nc.scalar.copy(
                                out=xT[key][cc][:, i * 128:(i + 1) * 128],
                                in_=ps[:, cc * 128:(cc + 1) * 128])

                # ---------- LN stats (mu, 1/rstd, rstd as [1, n] rows) -----
                rows = {}
                for key in ("q", "k"):
                    mu_ps = ps_row.tile([1, n], FP32, tag="row")
                    for cc in range(ncc):
                        for b in range(nb):
                            nc.tensor.matmul(
                                mu_ps[:, b * 512:(b + 1) * 512],
                                lhsT=ones_col.bitcast(F32R),
                                rhs=xT[key][cc][:, b * 512:(b + 1) * 512]
                                .bitcast(F32R),
                                start=(cc == 0), stop=(cc == ncc - 1))
                    mu_row = rows_t[R_MU[key]:R_MU[key] + 1, :]
                    nc.scalar.activation(mu_row, mu_ps, AF.Copy,
                                         scale=1.0 / DM)

                    m2_ps = ps_row.tile([1, n], FP32, tag="row")
                    for cc in range(ncc):
                        xsq = sqp.tile([128, n], FP32, tag="xsq")
                        nc.gpsimd.tensor_mul(xsq, xT[key][cc], xT[key][cc])
                        for b in range(nb):
                            nc.tensor.matmul(
                                m2_ps[:, b * 512:(b + 1) * 512],
                                lhsT=ones_col.bitcast(F32R),
                                rhs=xsq[:, b * 512:(b + 1) * 512]
                                .bitcast(F32R),
                                start=(cc == 0), stop=(cc == ncc - 1))
                    var_row = rows_t[6:7, :]
                    nc.scalar.activation(var_row, m2_ps, AF.Copy,
                                         scale=1.0 / DM)
                    musq = rows_t[7:8, :]
                    nc.vector.tensor_mul(musq, mu_row, mu_row)
                    nc.vector.tensor_sub(var_row, var_row, musq)
                    irs_row = rows_t[R_IRS[key]:R_IRS[key] + 1, :]
                    nc.scalar.activation(irs_row, var_row, AF.Sqrt,
                                         bias=eps_t)
                    rstd_row = rows_t[R_RSTD[key]:R_RSTD[key] + 1, :]
                    nc.vector.reciprocal(rstd_row, irs_row)
                    rows[key] = (mu_row, irs_row, rstd_row)

                # rstd_q per-partition [128, nt] (DMA shuffle), * 0.125
                def row_to_pp(row, dst):
                    for t in range(nt):
                        nc.sync.dma_start(
                            out=dst[:, t:t + 1],
                            in_=row[0:1, t * 128:(t + 1) * 128])

                sc_pp = small.tile([128, nt], FP32, tag="scpp")
                row_to_pp(rows["q"][2], sc_pp)
                nc.vector.tensor_scalar_mul(sc_pp, sc_pp, 0.125)
                rstdk_pp = small.tile([128, nt], FP32, tag="rkpp")
                row_to_pp(rows["k"][2], rstdk_pp)

                # rstd broadcast tiles [DK, n] for the k / gate folds
                def row_bcast(row, tag):
                    out_t = p0tmp.tile([DK, n], FP32, tag=tag, name=tag)
                    for b in range(nb):
                        psb = ps0.tile([128, 512], FP32, tag="ps0")
                        nc.tensor.matmul(
                            psb[:DK, :], lhsT=ones_row[:, :DK],
                            rhs=row[:, b * 512:(b + 1) * 512])
                        nc.scalar.copy(out=out_t[:, b * 512:(b + 1) * 512],
                                       in_=psb[:DK, :])
                    return out_t

                rstd_kB = row_bcast(rows["k"][2], "rkB")
                rstd_qB = row_bcast(rows["q"][2], "rqB")

                # ---------- projections ----------
                def project(key, xkey, apply):
                    """[DK, n] psum = W'^T x^T - wbar (x) mu + beta (x) irs"""
                    wt = wT[key]
                    mu_row, irs_row, _ = rows[xkey]
                    wp = p0tmp.tile([128, ncc, DK], FP32, tag="wp",
                                    name=f"wp{key}")
                    for cc in range(ncc):
                        nc.vector.tensor_scalar_mul(
                            wp[:, cc, :], wt[:, cc, :], lnw_pp[:, cc:cc + 1])
                    wbar_ps = ps0.tile([128, 512], FP32, tag="ps0")
                    beta_ps = ps0.tile([128, 512], FP32, tag="ps0")
                    for cc in range(ncc):
                        nc.tensor.matmul(
                            wbar_ps[:1, :DK], lhsT=ones_col.bitcast(F32R),
                            rhs=wp[:, cc, :].bitcast(F32R),
                            start=(cc == 0), stop=(cc == ncc - 1))
                        nc.tensor.matmul(
                            beta_ps[:1, :DK],
                            lhsT=lnb_pp[:, cc:cc + 1].bitcast(F32R),
                            rhs=wt[:, cc, :].bitcast(F32R),
                            start=(cc == 0), stop=(cc == ncc - 1))
                    nwbar = p0tmp.tile([1, DK], FP32, tag="nwbar",
                                       name=f"nwbar{key}")
                    nc.scalar.activation(nwbar, wbar_ps[:1, :DK], AF.Copy,
                                         scale=-1.0)
                    beta = p0tmp.tile([1, DK], FP32, tag="beta",
                                      name=f"beta{key}")
                    nc.scalar.copy(out=beta, in_=beta_ps[:1, :DK])

                    proj_ps = ps_row.tile([DK, n], FP32, tag="row")
                    for b in range(nb):
                        sl = slice(b * 512, (b + 1) * 512)
                        for cc in range(ncc):
                            nc.tensor.matmul(
                                proj_ps[:, sl],
                                lhsT=wp[:, cc, :].bitcast(F32R),
                                rhs=xT[xkey][cc][:, sl].bitcast(F32R),
                                start=(cc == 0), stop=False)
                        nc.tensor.matmul(
                            proj_ps[:, sl], lhsT=nwbar.bitcast(F32R),
                            rhs=mu_row[:, sl].bitcast(F32R),
                            start=False, stop=False)
                        nc.tensor.matmul(
                            proj_ps[:, sl], lhsT=beta.bitcast(F32R),
                            rhs=irs_row[:, sl].bitcast(F32R),
                            start=False, stop=True)
                    return apply(proj_ps)

                def to_sbuf(tag):
                    def f(ps):
                        pool = small if tag == "q1aT" else p0tmp
                        t = pool.tile([DK, n], FP32, tag=tag, name=tag)
                        nc.scalar.copy(out=t, in_=ps)
                        return t
                    return f

                q1aT = project("q", "q", to_sbuf("q1aT"))

                def k_apply(ps):
                    t = small.tile([DK, n], F32R, tag="k1T")
                    nc.vector.tensor_mul(t, ps, rstd_kB)
                    return t
                k1T = project("k", "k", k_apply)

                v1aT = project("v", "k", to_sbuf("v1aT"))

                def g_apply(ps):
                    t = p0tmp.tile([DK, n], FP32, tag="gpre", name="gpre")
                    nc.vector.tensor_mul(t, ps, rstd_qB)
                    g = p0tmp.tile([DK, n], FP32, tag="gateT", name="gateT")
                    nc.scalar.activation(g, t, AF.Sigmoid, bias=bg_t)
                    return g
                gateT = project("g", "q", g_apply)

                # v1 in k-partition layout [128, nt, DK] bf16, rstd_k applied
                v1 = small.tile([128, nt, DK], BF16, tag="v1")
                for i in range(nt):
                    psv = ps0.tile([128, 512], FP32, tag="ps0")
                    nc.tensor.transpose(
                        psv[:, :DK], v1aT[:, i * 128:(i + 1) * 128],
                        ident[:DK, :DK])
                    nc.vector.tensor_scalar_mul(
                        v1[:, i, :], psv[:, :DK], rstdk_pp[:, i:i + 1])

                # gate in q-partition layout [128, nt, DK] f32
                gate_qp = small.tile([128, nt, DK], FP32, tag="gateqp")
                for i in range(nt):
                    psg = ps0.tile([128, 512], FP32, tag="ps0")
                    nc.tensor.transpose(
                        psg[:, :DK], gateT[:, i * 128:(i + 1) * 128],
                        ident[:DK, :DK])
                    nc.scalar.copy(out=gate_qp[:, i, :], in_=psg[:, :DK])

                # qk_rel coefficients [128, nt, PK] (a-form, pre-rstd)
                qk5 = small.tile([128, nt, PK], FP32, tag="qk5")
                for i in range(nt):
                    ps5 = ps0.tile([128, 512], FP32, tag="ps0")
                    nc.tensor.matmul(
                        ps5[:, :PK],
                        lhsT=q1aT[:, i * 128:(i + 1) * 128].bitcast(FP32),
                        rhs=relT)
                    nc.vector.tensor_scalar_add(qk5[:, i, :], ps5[:, :PK],
                                                SHIFT)

            # ========= phase A: logits -> T (bf16 slab), row/col sums =======
            T_tiles = []
            with (
                tc.tile_pool(name="gidx", bufs=4) as gidxp,
                tc.tile_pool(name="lut", bufs=4) as lutp,
                tc.tile_pool(name="ps_logit", bufs=1, space="PSUM") as ps_logit,
                tc.tile_pool(name="ps_cs", bufs=1, space="PSUM") as ps_cs,
            ):
                rs_all = small.tile([128, nt], FP32, tag="rs")
                cs_ps = ps_cs.tile([128, n], FP32)
                for i in range(nt):
                    g_t = gidxp.tile([128, n], I8, tag="g")
                    nc.sync.dma_start(
                        out=g_t, in_=gidx_d[i * 128:(i + 1) * 128, :])
                    lps = ps_logit.tile([128, n], FP32, tag="lg")
                    for b in range(nb):
                        sl = slice(b * 512, (b + 1) * 512)
                        nc.tensor.matmul(
                            lps[:, sl],
                            lhsT=q1aT[:, i * 128:(i + 1) * 128].bitcast(F32R),
                            rhs=k1T[:, sl].bitcast(F32R),
                            start=True, stop=False)
                    lutA = lutp.tile([128, n], FP32, tag="lut")
                    nc.vector._custom_dve(
                        LUT_A, out=lutA, in0=g_t, in1=qk5[:, i, 4:5],
                        s0=qk5[:, i, 2:3], s1=qk5[:, i, 3:4])
                    lutB = lutp.tile([128, n], FP32, tag="lut")
                    nc.vector._custom_dve(
                        LUT_B, out=lutB, in0=g_t, in1=lutA,
                        s0=qk5[:, i, 1:2], s1=qk5[:, i, 0:1])
                    for b in range(nb):
                        sl = slice(b * 512, (b + 1) * 512)
                        nc.tensor.matmul(
                            lps[:, sl], lhsT=ident.bitcast(F32R),
                            rhs=lutB[:, sl].bitcast(F32R),
                            start=False, stop=True)
                    T_t = tslab.tile([128, n], BF16, tag=f"T{i}")
                    nc.scalar.activation(
                        T_t, lps, AF.Exp, scale=sc_pp[:, i:i + 1],
                        bias=nbias[:, i:i + 1],
                        accum_out=rs_all[:, i:i + 1])
                    T_tiles.append(T_t)
                    for b in range(nb):
                        sl = slice(b * 512, (b + 1) * 512)
                        nc.tensor.matmul(
                            cs_ps[:, sl], lhsT=ones_colb,
                            rhs=T_t[:, sl], start=(i == 0),
                            stop=(i == nt - 1))

                rinv = small.tile([128, nt], FP32, tag="rinv")
                nc.vector.reciprocal(rinv, rs_all)
                cinvB = small.tile([128, n], FP32, tag="cinvB")
                nc.vector.reciprocal(cinvB, cs_ps)

            # sqrt(cinv) in per-partition form [128, nt]
            sqc_row = rows_t[8:9, :]
            nc.scalar.activation(sqc_row, cinvB[:1, :], AF.Sqrt)
            sqc_pp = small.tile([128, nt], FP32, tag="sqcpp")
            for t in range(nt):
                nc.sync.dma_start(
                    out=sqc_pp[:, t:t + 1],
                    in_=sqc_row[0:1, t * 128:(t + 1) * 128])

            # ========= phase B: attn out, attn @ v, gate, Wo ===============
            with (
                tc.tile_pool(name="attnout", bufs=3) as attnp,
                tc.tile_pool(name="t2t", bufs=3) as t2tp,
                tc.tile_pool(name="fin", bufs=3) as finp,
                tc.tile_pool(name="ps_tt", bufs=2, space="PSUM") as ps_tt,
                tc.tile_pool(name="ps_av", bufs=2, space="PSUM") as ps_av,
                tc.tile_pool(name="ps_og", bufs=2, space="PSUM") as ps_og,
            ):
                ogT = small.tile([DK, n], FP32, tag="ogT")
                for i in range(nt):
                    a_t = attnp.tile([128, n], FP32, tag="a")
                    nc.vector._custom_dve(
                        COMBINE, out=a_t, in0=T_tiles[i], in1=cinvB,
                        s0=rinv[:, i:i + 1])
                    nc.sync.dma_start(
                        out=attn_d[i * 128:(i + 1) * 128, :], in_=a_t)
                    # AV: out_g[q, :] = rinv[q] * sum_k (T*sqrt(cinv))^2 v1[k]
                    avp = ps_av.tile([128, DK], FP32, tag="av")
                    for jj in range(nt // 4):
                        tt = ps_tt.tile([128, 512], BF16, tag="tt")
                        for u in range(4):
                            j = jj * 4 + u
                            nc.tensor.transpose(
                                tt[:, u * 128:(u + 1) * 128],
                                T_tiles[i][:, j * 128:(j + 1) * 128], identb)
                        t2 = t2tp.tile([128, 512], BF16, tag="t2")
                        for u in range(4):
                            j = jj * 4 + u
                            nc.scalar.activation(
                                t2[:, u * 128:(u + 1) * 128],
                                tt[:, u * 128:(u + 1) * 128],
                                AF.Square, scale=sqc_pp[:, j:j + 1])
                        for u in range(4):
                            j = jj * 4 + u
                            nc.tensor.matmul(
                                avp, lhsT=t2[:, u * 128:(u + 1) * 128],
                                rhs=v1[:, j, :],
                                start=(j == 0), stop=(j == nt - 1))
                    og = finp.tile([128, DK], FP32, tag="og")
                    nc.vector.scalar_tensor_tensor(
                        out=og, in0=avp, scalar=rinv[:, i:i + 1],
                        in1=gate_qp[:, i, :], op0=ALU.mult, op1=ALU.mult)
                    ogp_ps = ps_og.tile([128, 128], FP32, tag="ogt")
                    nc.tensor.transpose(ogp_ps[:DK, :], og, ident)
                    nc.scalar.copy(
                        out=ogT[:, i * 128:(i + 1) * 128],
                        in_=ogp_ps[:DK, :])

                # ---------- AllGather heads + output projection ----------
                nc.sync.dma_start(out=cc_in[:, :], in_=ogT)
                nc.gpsimd.collective_compute(
                    "AllGather", ALU.bypass,
                    replica_groups=[list(range(H))],
                    ins=[cc_in[:, :]], outs=[cc_out[:, :]])
                GT = []
                for cc in range(ncc):
                    t = small.tile([128, n], FP32, tag=f"gt{cc}")
                    nc.sync.dma_start(
                        out=t, in_=cc_out[cc * 128:(cc + 1) * 128, :])
                    GT.append(t)
                for i in range(nt):
                    ps_o = ps_og.tile([128, DM], FP32, tag="fin")
                    for cc in range(ncc):
                        nc.tensor.matmul(
                            ps_o,
                            lhsT=GT[cc][:, i * 128:(i + 1) * 128]
                            .bitcast(F32R),
                            rhs=woT[:, cc, :].bitcast(F32R),
                            start=(cc == 0), stop=False)
                    nc.tensor.matmul(
                        ps_o, lhsT=ones_row.bitcast(F32R),
                        rhs=bo_t.bitcast(F32R), start=False, stop=True)
                    o_t = finp.tile([128, DM], FP32, tag="fout")
                    nc.scalar.copy(out=o_t, in_=ps_o)
                    nc.sync.dma_start(
                        out=out_d[i * 128:(i + 1) * 128, :], in_=o_t)

    nc.compile()
    _NC_CACHE[n] = nc
    return nc


# --------------------------------------------------------------------------
# Host entry: shard, run SPMD on 8 cores, gather.
# --------------------------------------------------------------------------
def make_in_maps(x_q, x_k, gpm, mask, ln_w, ln_b, Wq, Wk, Wv, Wg, bg, Wo, bo,
                 rel_k):
    f32 = np.float32
    gidx = np.where(np.asarray(mask, bool),
                    np.asarray(gpm) - 2, 64).astype(np.int8)
    base = {
        "x_q": np.ascontiguousarray(np.asarray(x_q, f32)),
        "x_k": np.ascontiguousarray(np.asarray(x_k, f32)),
        "gidx": gidx,
        "woT": np.ascontiguousarray(np.asarray(Wo, f32).T),
        "rel_kT": np.ascontiguousarray(np.asarray(rel_k, f32).T),
        "ln_w": np.ascontiguousarray(np.asarray(ln_w, f32)),
        "ln_b": np.ascontiguousarray(np.asarray(ln_b, f32)),
        "bo": np.ascontiguousarray(np.asarray(bo, f32).reshape(1, DM)),
    }
    in_maps = []
    for h in range(H):
        sl = slice(h * DK, (h + 1) * DK)
        m = dict(base)
        m["wqT"] = np.ascontiguousarray(np.asarray(Wq, f32)[sl].T)
        m["wkT"] = np.ascontiguousarray(np.asarray(Wk, f32)[sl].T)
        m["wvT"] = np.ascontiguousarray(np.asarray(Wv, f32)[sl].T)
        m["wgT"] = np.ascontiguousarray(np.asarray(Wg, f32)[sl].T)
        m["bg_h"] = np.ascontiguousarray(
            np.asarray(bg, f32)[sl].reshape(DK, 1))
        in_maps.append(m)
    return in_maps


def kernel(**inputs):
    from concourse.bass_utils import run_bass_kernel_spmd

    nc = build_nc(N_FULL)
    in_maps = make_in_maps(**inputs)
    res = run_bass_kernel_spmd(nc, in_maps, list(range(H)))
    out = np.asarray(res.results[0]["out"], np.float32)
    attn = np.stack(
        [np.asarray(res.results[h]["attn"], np.float32) for h in range(H)], 0)
    return out, attn


# revision 18
# speedup vs baseline: 1.0943x; 1.0943x over previous
"""Trainium2 Bass kernel for nn_MultiHeadAttention_39883066311260.

Sparse multi-head attention with relative-position-bucket bias and a
double softmax (row-softmax * column-softmax), sharded one head per
NeuronCore across 8 cores (tensor parallel on the head dim).

Math (per head h, all on device):
    xq = LN(x_q); xk = LN(x_k)            (LN folded into the projections)
    q = xq Wq_h^T, k = xk Wk_h^T, v = xk Wv_h^T
    S = (q k^T + qk_rel[q, gpm]) / 8, masked -> -1e9-ish
    attn = softmax_row(S) * softmax_col(S) = T^2 * rinv[q] * cinv[k]
        with T = exp(S) (unstabilized; logits are O(10), masked -> exp=0)
    out_h = attn @ v;  gated; AllGather heads; out = G Wo^T + bo

The per-element bucket gather qk_rel[q, gpm[q,k]] runs as two custom DVE
ops (per-partition LUT via sum-of-exclusive-products over an int8-coded
index in {-2..2}; mask folded into the index on the host as code 64,
which matches no bucket and nets a -SHIFT logit via the exp bias).
"""

import numpy as np

import concourse.bass as bass
import concourse.bacc as bacc
import concourse.mybir as mybir
import concourse.tile as tile

H, DM, DK, PK = 8, 512, 64, 5
N_FULL = 2048
FP32 = mybir.dt.float32
F32R = mybir.dt.float32r
BF16 = mybir.dt.bfloat16
I8 = mybir.dt.int8
AF = mybir.ActivationFunctionType
ALU = mybir.AluOpType
SHIFT = 2048.0
EPS = 1e-6

# --------------------------------------------------------------------------
# Custom DVE ops: 6-entry per-partition LUT in two passes + fused combine.
# --------------------------------------------------------------------------
_DVE_OPS = {}


def _register_dve_ops():
    global _DVE_OPS
    if _DVE_OPS:
        return _DVE_OPS
    import concourse.dve_ops as D
    from concourse.dve_spec import (
        C0, C1, C2, C3, One, Spec, Src0, Src1, Zero, eq, sq, lower,
        _spill_c3_to_src1, _has_src1,
    )
    from concourse.dve_uop import DveOpSpec

    if any(op.name == "MHA_LUT_A" for op in D.OPS):
        _DVE_OPS = {op.name: op for op in D.OPS if op.name.startswith("MHA_")}
        return _DVE_OPS

    TWO = One + One
    NEG1 = Zero - One
    NEG2 = NEG1 + NEG1

    def build(name, spec):
        row = D._CUSTOM_DVE_ROW_BASE + len(D.OPS)
        D._SUB_OPCODE_FOR_NAME[name] = row
        shas = {}
        for ver in ("v3", "v4"):
            s = DveOpSpec(name=name, opcode=row, uops=lower(spec, ver=ver),
                          rd1_en=_has_src1(spec))
            shas[ver] = s.sha(ver)
        op = D.DveOp(name, spec, subdim=False, uops_sha=shas)
        D.OPS.append(op)
        D.CUSTOM_DVE_SPECS[name] = spec
        _DVE_OPS[name] = op
        return op

    # gidx is int8-encoded: bucket b -> b-2 in {-2..2}, masked -> 64.
    # Table values arrive pre-shifted by +SHIFT; the exp pass bias subtracts
    # SHIFT, so an unmatched index (masked) nets -SHIFT = a huge negative
    # logit whose exp underflows to 0, with no explicit mask compare.
    # pass A: encoded {0,1,2} = buckets {2,3,4} via s0/s1/in1(C3-spill).
    body_a = (eq(Src0, Zero) * C0 + eq(Src0, One) * C1
              + eq(Src0, TWO) * C3)
    build("MHA_LUT_A", Spec(
        body=_spill_c3_to_src1(body_a),
        reference=lambda in0, in1, s0, s1, imm2: (
            (in0 == 0) * s0 + (in0 == 1) * s1
            + (in0 == 2) * in1).astype(np.float32),
    ))
    # pass B: encoded {-1,-2} = buckets {1,0} added onto pass A's output.
    body_b = Src1 + eq(Src0, NEG1) * C0 + eq(Src0, NEG2) * C1
    build("MHA_LUT_B", Spec(
        body=body_b,
        reference=lambda in0, in1, s0, s1, imm2: (
            in1 + (in0 == -1) * s0 + (in0 == -2) * s1).astype(np.float32),
    ))
    # combine: attn = T^2 * rinv[q] * cinvB
    build("MHA_COMBINE", Spec(
        body=sq(Src0) * C0 * Src1,
        reference=lambda in0, in1, s0, s1, imm2: (
            in0.astype(np.float32) ** 2 * s0 * in1).astype(np.float32),
    ))
    return _DVE_OPS


# --------------------------------------------------------------------------
# The per-core Bass program (SPMD: identical program, per-core input values).
# --------------------------------------------------------------------------
_NC_CACHE = {}


def build_nc(n=N_FULL):
    if n in _NC_CACHE:
        return _NC_CACHE[n]
    ops = _register_dve_ops()
    LUT_A, LUT_B, COMBINE = ops["MHA_LUT_A"], ops["MHA_LUT_B"], ops["MHA_COMBINE"]
    from concourse.masks import make_identity

    nt = n // 128          # q/k tiles of 128
    nb = n // 512          # 512-wide PSUM bank chunks
    ncc = DM // 128        # channel chunks

    nc = bacc.Bacc("TRN2", num_devices=H)

    # ---- I/O ----
    xq_d = nc.dram_tensor("x_q", [n, DM], FP32, kind="ExternalInput")
    xk_d = nc.dram_tensor("x_k", [n, DM], FP32, kind="ExternalInput")
    gidx_d = nc.dram_tensor("gidx", [n, n], I8, kind="ExternalInput")
    wqT_d = nc.dram_tensor("wqT", [DM, DK], FP32, kind="ExternalInput")
    wkT_d = nc.dram_tensor("wkT", [DM, DK], FP32, kind="ExternalInput")
    wvT_d = nc.dram_tensor("wvT", [DM, DK], FP32, kind="ExternalInput")
    wgT_d = nc.dram_tensor("wgT", [DM, DK], FP32, kind="ExternalInput")
    woT_d = nc.dram_tensor("woT", [DM, DM], FP32, kind="ExternalInput")
    relT_d = nc.dram_tensor("rel_kT", [DK, PK], FP32, kind="ExternalInput")
    lnw_d = nc.dram_tensor("ln_w", [DM], FP32, kind="ExternalInput")
    lnb_d = nc.dram_tensor("ln_b", [DM], FP32, kind="ExternalInput")
    bg_d = nc.dram_tensor("bg_h", [DK, 1], FP32, kind="ExternalInput")
    bo_d = nc.dram_tensor("bo", [1, DM], FP32, kind="ExternalInput")

    attn_d = nc.dram_tensor("attn", [n, n], FP32, kind="ExternalOutput")
    out_d = nc.dram_tensor("out", [n, DM], FP32, kind="ExternalOutput")

    cc_in = nc.dram_tensor("cc_in", [DK, n], FP32)
    cc_out = nc.dram_tensor("cc_out", [DM, n], FP32, addr_space="Shared")

    with tile.TileContext(nc) as tc:
        with (
            tc.tile_pool(name="const", bufs=1) as const,
            tc.tile_pool(name="small", bufs=1) as small,
            tc.tile_pool(name="tslab", bufs=1) as tslab,
        ):
            # ---------- constants ----------
            ident = const.tile([128, 128], FP32)
            make_identity(nc, ident)
            identb = const.tile([128, 128], BF16)
            nc.vector.tensor_copy(identb, ident)
            ones_col = const.tile([128, 1], FP32)
            nc.vector.memset(ones_col, 1.0)
            ones_colb = const.tile([128, 128], BF16)
            nc.vector.memset(ones_colb, 1.0)
            ones_row = const.tile([1, 128], FP32)
            nc.vector.memset(ones_row, 1.0)
            ones_colr = const.tile([128, 1], F32R)
            nc.vector.tensor_copy(ones_colr, ones_col)
            identr = const.tile([128, 128], F32R)
            nc.vector.tensor_copy(identr, ident)
            ones_rowb = const.tile([1, 128], BF16)
            nc.vector.tensor_copy(ones_rowb, ones_row)
            eps_t = const.tile([1, 1], FP32)
            nc.vector.memset(eps_t, EPS)
            eps128 = const.tile([128, 1], FP32)
            nc.vector.memset(eps128, EPS)

            relT = const.tile([DK, PK], FP32)
            nc.sync.dma_start(out=relT, in_=relT_d[:, :])
            bg_t = const.tile([DK, 1], FP32)
            nc.sync.dma_start(out=bg_t, in_=bg_d[:, :])
            bo_t = const.tile([1, DM], FP32)
            nc.sync.dma_start(out=bo_t, in_=bo_d[:, :])
            # ln_w / ln_b in per-partition chunk form [128, ncc]
            lnw_pp = const.tile([128, ncc], FP32)
            nc.sync.dma_start(
                out=lnw_pp, in_=lnw_d.rearrange("(cc p) -> p cc", p=128))
            lnb_pp = const.tile([128, ncc], FP32)
            nc.sync.dma_start(
                out=lnb_pp, in_=lnb_d.rearrange("(cc p) -> p cc", p=128))

            wT = {}
            for name, dram in (("q", wqT_d), ("k", wkT_d), ("v", wvT_d),
                               ("g", wgT_d)):
                t = const.tile([128, ncc, DK], FP32, tag=f"w{name}")
                nc.sync.dma_start(
                    out=t, in_=dram.rearrange("(cc p) d -> p cc d", p=128))
                wT[name] = t
            woT = const.tile([128, ncc, DM], FP32)
            nc.sync.dma_start(
                out=woT, in_=woT_d.rearrange("(cc p) d -> p cc d", p=128))

            # ================= phase 0: LN + projections =================
            with (
                tc.tile_pool(name="xload", bufs=3) as xload,
                tc.tile_pool(name="xt", bufs=1) as xtp,
                tc.tile_pool(name="sqbuf", bufs=2) as sqp,
                tc.tile_pool(name="p0tmp", bufs=1) as p0tmp,
                tc.tile_pool(name="ps_row", bufs=1, space="PSUM") as ps_row,
                tc.tile_pool(name="ps0", bufs=2, space="PSUM") as ps0,
            ):
                rows_t = small.tile([9, n], FP32, tag="rows")
                R_MU = {"q": 0, "k": 3}
                R_IRS = {"q": 1, "k": 4}
                R_RSTD = {"q": 2, "k": 5}
                # ---------- load x and transpose to [DM, n] ----------
                xT = {}
                for key, dram in (("q", xq_d), ("k", xk_d)):
                    tiles = [xtp.tile([128, n], F32R, tag=f"xt{key}{cc}",
                                      name=f"xt{key}{cc}")
                             for cc in range(ncc)]
                    xT[key] = tiles
                    for i in range(nt):
                        xt_in = xload.tile([128, DM], FP32, tag="xin")
                        nc.sync.dma_start(
                            out=xt_in, in_=dram[i * 128:(i + 1) * 128, :])
                        ps = ps0.tile([128, 512], FP32, tag="ps0")
                        for cc in range(ncc):
                            nc.tensor.transpose(
                                ps[:, cc * 128:(cc + 1) * 128],
                                xt_in[:, cc * 128:(cc + 1) * 128], ident)
                        for cc in range(ncc):
                            nc.scalar.copy(
                                out=xT[key][cc][:, i * 128:(i + 1) * 128],
                                in_=ps[:, cc * 128:(cc + 1) * 128])

                # ---------- LN stats (mu, 1/rstd, rstd as [1, n] rows) -----
                rows = {}
                for key in ("q", "k"):
                    mu_ps = ps_row.tile([1, n], FP32, tag="row")
                    for cc in range(ncc):
                        for b in range(nb):
                            nc.tensor.matmul(
                                mu_ps[:, b * 512:(b + 1) * 512],
                                lhsT=ones_col.bitcast(F32R),
                                rhs=xT[key][cc][:, b * 512:(b + 1) * 512]
                                .bitcast(F32R),
                                start=(cc == 0), stop=(cc == ncc - 1))
                    mu_row = rows_t[R_MU[key]:R_MU[key] + 1, :]
                    nc.scalar.activation(mu_row, mu_ps, AF.Copy,
                                         scale=1.0 / DM)

                    m2_ps = ps_row.tile([1, n], FP32, tag="row")
                    for cc in range(ncc):
                        xsq = sqp.tile([128, n], FP32, tag="xsq")
                        nc.gpsimd.tensor_mul(xsq, xT[key][cc], xT[key][cc])
                        for b in range(nb):
                            nc.tensor.matmul(
                                m2_ps[:, b * 512:(b + 1) * 512],
                                lhsT=ones_col.bitcast(F32R),
                                rhs=xsq[:, b * 512:(b + 1) * 512]
                                .bitcast(F32R),
                                start=(cc == 0), stop=(cc == ncc - 1))
                    var_row = rows_t[6:7, :]
                    nc.scalar.activation(var_row, m2_ps, AF.Copy,
                                         scale=1.0 / DM)
                    musq = rows_t[7:8, :]
                    nc.vector.tensor_mul(musq, mu_row, mu_row)
                    nc.vector.tensor_sub(var_row, var_row, musq)
                    irs_row = rows_t[R_IRS[key]:R_IRS[key] + 1, :]
                    nc.scalar.activation(irs_row, var_row, AF.Sqrt,
                                         bias=eps_t)
                    rstd_row = rows_t[R_RSTD[key]:R_RSTD[key] + 1, :]
                    nc.vector.reciprocal(rstd_row, irs_row)
                    rows[key] = (mu_row, irs_row, rstd_row)

                # rstd_q per-partition [128, nt] (DMA shuffle), * 0.125
                def row_to_pp(row, dst):
                    for t in range(nt):
                        nc.sync.dma_start(
                            out=dst[:, t:t + 1],
                            in_=row[0:1, t * 128:(t + 1) * 128])

                sc_pp = small.tile([128, nt], FP32, tag="scpp")
                row_to_pp(rows["q"][2], sc_pp)
                nc.vector.tensor_scalar_mul(sc_pp, sc_pp, 0.125)
                rstdk_pp = small.tile([128, nt], FP32, tag="rkpp")
                row_to_pp(rows["k"][2], rstdk_pp)

                # rstd broadcast tiles [DK, n] for the k / gate folds
                def row_bcast(row, tag):
                    out_t = p0tmp.tile([DK, n], FP32, tag=tag, name=tag)
                    for b in range(nb):
                        psb = ps0.tile([128, 512], FP32, tag="ps0")
                        nc.tensor.matmul(
                            psb[:DK, :], lhsT=ones_row[:, :DK],
                            rhs=row[:, b * 512:(b + 1) * 512])
                        nc.scalar.copy(out=out_t[:, b * 512:(b + 1) * 512],
                                       in_=psb[:DK, :])
                    return out_t

                rstd_kB = row_bcast(rows["k"][2], "rkB")
                rstd_qB = row_bcast(rows["q"][2], "rqB")

                # ---------- projections ----------
                def project(key, xkey, apply):
                    """[DK, n] psum = W'^T x^T - wbar (x) mu + beta (x) irs"""
                    wt = wT[key]
                    mu_row, irs_row, _ = rows[xkey]
                    wp = p0tmp.tile([128, ncc, DK], FP32, tag="wp",
                                    name=f"wp{key}")
                    for cc in range(ncc):
                        nc.vector.tensor_scalar_mul(
                            wp[:, cc, :], wt[:, cc, :], lnw_pp[:, cc:cc + 1])
                    wbar_ps = ps0.tile([128, 512], FP32, tag="ps0")
                    beta_ps = ps0.tile([128, 512], FP32, tag="ps0")
                    for cc in range(ncc):
                        nc.tensor.matmul(
                            wbar_ps[:1, :DK], lhsT=ones_col.bitcast(F32R),
                            rhs=wp[:, cc, :].bitcast(F32R),
                            start=(cc == 0), stop=(cc == ncc - 1))
                        nc.tensor.matmul(
                            beta_ps[:1, :DK],
                            lhsT=lnb_pp[:, cc:cc + 1].bitcast(F32R),
                            rhs=wt[:, cc, :].bitcast(F32R),
                            start=(cc == 0), stop=(cc == ncc - 1))
                    nwbar = p0tmp.tile([1, DK], FP32, tag="nwbar",
                                       name=f"nwbar{key}")
                    nc.scalar.activation(nwbar, wbar_ps[:1, :DK], AF.Copy,
                                         scale=-1.0)
                    beta = p0tmp.tile([1, DK], FP32, tag="beta",
                                      name=f"beta{key}")
                    nc.scalar.copy(out=beta, in_=beta_ps[:1, :DK])

                    proj_ps = ps_row.tile([DK, n], FP32, tag="row")
                    for b in range(nb):
                        sl = slice(b * 512, (b + 1) * 512)
                        for cc in range(ncc):
                            nc.tensor.matmul(
                                proj_ps[:, sl],
                                lhsT=wp[:, cc, :].bitcast(F32R),
                                rhs=xT[xkey][cc][:, sl].bitcast(F32R),
                                start=(cc == 0), stop=False)
                        nc.tensor.matmul(
                            proj_ps[:, sl], lhsT=nwbar.bitcast(F32R),
                            rhs=mu_row[:, sl].bitcast(F32R),
                            start=False, stop=False)
                        nc.tensor.matmul(
                            proj_ps[:, sl], lhsT=beta.bitcast(F32R),
                            rhs=irs_row[:, sl].bitcast(F32R),
                            start=False, stop=True)
                    return apply(proj_ps)

                def to_sbuf(tag):
                    def f(ps):
                        pool = small if tag == "q1aT" else p0tmp
                        t = pool.tile([DK, n], FP32, tag=tag, name=tag)
                        nc.scalar.copy(out=t, in_=ps)
                        return t
                    return f

                q1aT = project("q", "q", to_sbuf("q1aT"))

                def k_apply(ps):
                    t = small.tile([DK, n], F32R, tag="k1T")
                    nc.vector.tensor_mul(t, ps, rstd_kB)
                    return t
                k1T = project("k", "k", k_apply)

                v1aT = project("v", "k", to_sbuf("v1aT"))

                def g_apply(ps):
                    t = p0tmp.tile([DK, n], FP32, tag="gpre", name="gpre")
                    nc.vector.tensor_mul(t, ps, rstd_qB)
                    g = p0tmp.tile([DK, n], FP32, tag="gateT", name="gateT")
                    nc.scalar.activation(g, t, AF.Sigmoid, bias=bg_t)
                    return g
                gateT = project("g", "q", g_apply)

                # v1 in k-partition layout [128, nt, DK] bf16, rstd_k applied
                v1 = small.tile([128, nt, DK], BF16, tag="v1")
                for i in range(nt):
                    psv = ps0.tile([128, 512], FP32, tag="ps0")
                    nc.tensor.transpose(
                        psv[:, :DK], v1aT[:, i * 128:(i + 1) * 128],
                        ident[:DK, :DK])
                    nc.vector.tensor_scalar_mul(
                        v1[:, i, :], psv[:, :DK], rstdk_pp[:, i:i + 1])

                # gate in q-partition layout [128, nt, DK] f32
                gate_qp = small.tile([128, nt, DK], FP32, tag="gateqp")
                for i in range(nt):
                    psg = ps0.tile([128, 512], FP32, tag="ps0")
                    nc.tensor.transpose(
                        psg[:, :DK], gateT[:, i * 128:(i + 1) * 128],
                        ident[:DK, :DK])
                    nc.scalar.copy(out=gate_qp[:, i, :], in_=psg[:, :DK])

                # qk_rel coefficients [128, nt, PK] (a-form, pre-rstd)
                qk5 = small.tile([128, nt, PK], FP32, tag="qk5")
                for i in range(nt):
                    ps5 = ps0.tile([128, 512], FP32, tag="ps0")
                    nc.tensor.matmul(
                        ps5[:, :PK],
                        lhsT=q1aT[:, i * 128:(i + 1) * 128].bitcast(FP32),
                        rhs=relT)
                    nc.vector.tensor_scalar_add(qk5[:, i, :], ps5[:, :PK],
                                                SHIFT)

            # ========= phase A: logits -> T (bf16 slab), row/col sums =======
            T_tiles = []
            with (
                tc.tile_pool(name="gidx", bufs=4) as gidxp,
                tc.tile_pool(name="lut", bufs=4) as lutp,
                tc.tile_pool(name="ps_logit", bufs=1, space="PSUM") as ps_logit,
                tc.tile_pool(name="ps_cs", bufs=1, space="PSUM") as ps_cs,
            ):
                rs_all = small.tile([128, nt], FP32, tag="rs")
                cs_ps = ps_cs.tile([128, n], FP32)
                for i in range(nt):
                    g_t = gidxp.tile([128, n], I8, tag="g")
                    nc.sync.dma_start(
                        out=g_t, in_=gidx_d[i * 128:(i + 1) * 128, :])
                    lps = ps_logit.tile([128, n], FP32, tag="lg")
                    for b in range(nb):
                        sl = slice(b * 512, (b + 1) * 512)
                        nc.tensor.matmul(
                            lps[:, sl],
                            lhsT=q1aT[:, i * 128:(i + 1) * 128].bitcast(F32R),
                            rhs=k1T[:, sl].bitcast(F32R),
                            start=True, stop=False)
                    lutA = lutp.tile([128, n], FP32, tag="lut")
                    nc.vector._custom_dve(
                        LUT_A, out=lutA, in0=g_t, in1=qk5[:, i, 4:5],
                        s0=qk5[:, i, 2:3], s1=qk5[:, i, 3:4])
                    lutB = lutp.tile([128, n], FP32, tag="lut")
                    nc.vector._custom_dve(
                        LUT_B, out=lutB, in0=g_t, in1=lutA,
                        s0=qk5[:, i, 1:2], s1=qk5[:, i, 0:1])
                    for b in range(nb):
                        sl = slice(b * 512, (b + 1) * 512)
                        nc.tensor.matmul(
                            lps[:, sl], lhsT=ident.bitcast(F32R),
                            rhs=lutB[:, sl].bitcast(F32R),
                            start=False, stop=True)
                    T_t = tslab.tile([128, n], BF16, tag=f"T{i}")
                    nc.scalar.activation(
                        T_t, lps, AF.Exp, scale=sc_pp[:, i:i + 1],
                        bias=nbias[:, i:i + 1],
                        accum_out=rs_all[:, i:i + 1])
                    T_tiles.append(T_t)
                    for b in range(nb):
                        sl = slice(b * 512, (b + 1) * 512)
                        nc.tensor.matmul(
                            cs_ps[:, sl], lhsT=ones_colb,
                            rhs=T_t[:, sl], start=(i == 0),
                            stop=(i == nt - 1))

                rinv = small.tile([128, nt], FP32, tag="rinv")
                nc.vector.reciprocal(rinv, rs_all)
                cinvB = small.tile([128, n], FP32, tag="cinvB")
                nc.vector.reciprocal(cinvB, cs_ps)

            # sqrt(cinv) in per-partition form [128, nt]
            sqc_row = rows_t[8:9, :]
            nc.scalar.activation(sqc_row, cinvB[:1, :], AF.Sqrt)
            sqc_pp = small.tile([128, nt], FP32, tag="sqcpp")
            for t in range(nt):
                nc.sync.dma_start(
                    out=sqc_pp[:, t:t + 1],
                    in_=sqc_row[0:1, t * 128:(t + 1) * 128])

            # ========= phase B: attn out, attn @ v, gate, Wo ===============
            with (
                tc.tile_pool(name="attnout", bufs=3) as attnp,
                tc.tile_pool(name="t2t", bufs=3) as t2tp,
                tc.tile_pool(name="fin", bufs=3) as finp,
                tc.tile_pool(name="ps_tt", bufs=2, space="PSUM") as ps_tt,
                tc.tile_pool(name="ps_av", bufs=2, space="PSUM") as ps_av,
                tc.tile_pool(name="ps_og", bufs=2, space="PSUM") as ps_og,
            ):
                ogT = small.tile([DK, n], FP32, tag="ogT")
                for i in range(nt):
                    a_t = attnp.tile([128, n], FP32, tag="a")
                    nc.vector._custom_dve(
                        COMBINE, out=a_t, in0=T_tiles[i], in1=cinvB,
                        s0=rinv[:, i:i + 1])
                    nc.sync.dma_start(
                        out=attn_d[i * 128:(i + 1) * 128, :], in_=a_t)
                    # AV: out_g[q, :] = rinv[q] * sum_k (T*sqrt(cinv))^2 v1[k]
                    avp = ps_av.tile([128, DK], FP32, tag="av")
                    for jj in range(nt // 4):
                        tt = ps_tt.tile([128, 512], BF16, tag="tt")
                        for u in range(4):
                            j = jj * 4 + u
                            nc.tensor.transpose(
                                tt[:, u * 128:(u + 1) * 128],
                                T_tiles[i][:, j * 128:(j + 1) * 128], identb)
                        t2 = t2tp.tile([128, 512], BF16, tag="t2")
                        for u in range(4):
                            j = jj * 4 + u
                            nc.scalar.activation(
                                t2[:, u * 128:(u + 1) * 128],
                                tt[:, u * 128:(u + 1) * 128],
                                AF.Square, scale=sqc_pp[:, j:j + 1])
                        for u in range(4):
                            j = jj * 4 + u
                            nc.tensor.matmul(
                                avp, lhsT=t2[:, u * 128:(u + 1) * 128],
                                rhs=v1[:, j, :],
                                start=(j == 0), stop=(j == nt - 1))
                    og = finp.tile([128, DK], FP32, tag="og")
                    nc.vector.scalar_tensor_tensor(
                        out=og, in0=avp, scalar=rinv[:, i:i + 1],
                        in1=gate_qp[:, i, :], op0=ALU.mult, op1=ALU.mult)
                    ogp_ps = ps_og.tile([128, 128], FP32, tag="ogt")
                    nc.tensor.transpose(ogp_ps[:DK, :], og, ident)
                    nc.scalar.copy(
                        out=ogT[:, i * 128:(i + 1) * 128],
                        in_=ogp_ps[:DK, :])

                # ---------- AllGather heads + output projection ----------
                nc.sync.dma_start(out=cc_in[:, :], in_=ogT)
                nc.gpsimd.collective_compute(
                    "AllGather", ALU.bypass,
                    replica_groups=[list(range(H))],
                    ins=[cc_in[:, :]], outs=[cc_out[:, :]])
                GT = []
                for cc in range(ncc):
                    t = small.tile([128, n], FP32, tag=f"gt{cc}")
                    nc.sync.dma_start(
                        out=t, in_=cc_out[cc * 128:(cc + 1) * 128, :])
                    GT.append(t)
                for i in range(nt):
                    ps_o = ps_og.tile([128, DM], FP32, tag="fin")
                    for cc in range(ncc):
                        nc.tensor.matmul(
                            ps_o,
                            lhsT=GT[cc][:, i * 128:(i + 1) * 128]
                            .bitcast(F32R),
                            rhs=woT[:, cc, :].bitcast(F32R),
                            start=(cc == 0), stop=False)
                    nc.tensor.matmul(
                        ps_o, lhsT=ones_row.bitcast(F32R),
                        rhs=bo_t.bitcast(F32R), start=False, stop=True)
                    o_t = finp.tile([128, DM], FP32, tag="fout")
                    nc.scalar.copy(out=o_t, in_=ps_o)
                    nc.sync.dma_start(
                        out=out_d[i * 128:(i + 1) * 128, :], in_=o_t)

    nc.compile()
    _NC_CACHE[n] = nc
    return nc


# --------------------------------------------------------------------------
# Host entry: shard, run SPMD on 8 cores, gather.
# --------------------------------------------------------------------------
def make_in_maps(x_q, x_k, gpm, mask, ln_w, ln_b, Wq, Wk, Wv, Wg, bg, Wo, bo,
                 rel_k):
    f32 = np.float32
    gidx = np.where(np.asarray(mask, bool),
                    np.asarray(gpm) - 2, 64).astype(np.int8)
    base = {
        "x_q": np.ascontiguousarray(np.asarray(x_q, f32)),
        "x_k": np.ascontiguousarray(np.asarray(x_k, f32)),
        "gidx": gidx,
        "woT": np.ascontiguousarray(np.asarray(Wo, f32).T),
        "rel_kT": np.ascontiguousarray(np.asarray(rel_k, f32).T),
        "ln_w": np.ascontiguousarray(np.asarray(ln_w, f32)),
        "ln_b": np.ascontiguousarray(np.asarray(ln_b, f32)),
        "bo": np.ascontiguousarray(np.asarray(bo, f32).reshape(1, DM)),
    }
    in_maps = []
    for h in range(H):
        sl = slice(h * DK, (h + 1) * DK)
        m = dict(base)
        m["wqT"] = np.ascontiguousarray(np.asarray(Wq, f32)[sl].T)
        m["wkT"] = np.ascontiguousarray(np.asarray(Wk, f32)[sl].T)
        m["wvT"] = np.ascontiguousarray(np.asarray(Wv, f32)[sl].T)
        m["wgT"] = np.ascontiguousarray(np.asarray(Wg, f32)[sl].T)
        m["bg_h"] = np.ascontiguousarray(
            np.asarray(bg, f32)[sl].reshape(DK, 1))
        in_maps.append(m)
    return in_maps


def kernel(**inputs):
    from concourse.bass_utils import run_bass_kernel_spmd

    nc = build_nc(N_FULL)
    in_maps = make_in_maps(**inputs)
    res = run_bass_kernel_spmd(nc, in_maps, list(range(H)))
    out = np.asarray(res.results[0]["out"], np.float32)
    attn = np.stack(
        [np.asarray(res.results[h]["attn"], np.float32) for h in range(H)], 0)
    return out, attn


# revision 19
# speedup vs baseline: 1.0978x; 1.0032x over previous
"""Trainium2 Bass kernel for nn_MultiHeadAttention_39883066311260.

Sparse multi-head attention with relative-position-bucket bias and a
double softmax (row-softmax * column-softmax), sharded one head per
NeuronCore across 8 cores (tensor parallel on the head dim).

Math (per head h, all on device):
    xq = LN(x_q); xk = LN(x_k)            (LN folded into the projections)
    q = xq Wq_h^T, k = xk Wk_h^T, v = xk Wv_h^T
    S = (q k^T + qk_rel[q, gpm]) / 8, masked -> -1e9-ish
    attn = softmax_row(S) * softmax_col(S) = T^2 * rinv[q] * cinv[k]
        with T = exp(S) (unstabilized; logits are O(10), masked -> exp=0)
    out_h = attn @ v;  gated; AllGather heads; out = G Wo^T + bo

The per-element bucket gather qk_rel[q, gpm[q,k]] runs as two custom DVE
ops (per-partition LUT via sum-of-exclusive-products over an int8-coded
index in {-2..2}; mask folded into the index on the host as code 64,
which matches no bucket and nets a -SHIFT logit via the exp bias).
"""

import numpy as np

import concourse.bass as bass
import concourse.bacc as bacc
import concourse.mybir as mybir
import concourse.tile as tile

H, DM, DK, PK = 8, 512, 64, 5
N_FULL = 2048
FP32 = mybir.dt.float32
F32R = mybir.dt.float32r
BF16 = mybir.dt.bfloat16
I8 = mybir.dt.int8
AF = mybir.ActivationFunctionType
ALU = mybir.AluOpType
SHIFT = 2048.0
EPS = 1e-6

# --------------------------------------------------------------------------
# Custom DVE ops: 6-entry per-partition LUT in two passes + fused combine.
# --------------------------------------------------------------------------
_DVE_OPS = {}


def _register_dve_ops():
    global _DVE_OPS
    if _DVE_OPS:
        return _DVE_OPS
    import concourse.dve_ops as D
    from concourse.dve_spec import (
        C0, C1, C2, C3, One, Spec, Src0, Src1, Zero, eq, sq, lower,
        _spill_c3_to_src1, _has_src1,
    )
    from concourse.dve_uop import DveOpSpec

    if any(op.name == "MHA_LUT_A" for op in D.OPS):
        _DVE_OPS = {op.name: op for op in D.OPS if op.name.startswith("MHA_")}
        return _DVE_OPS

    TWO = One + One
    NEG1 = Zero - One
    NEG2 = NEG1 + NEG1

    def build(name, spec):
        row = D._CUSTOM_DVE_ROW_BASE + len(D.OPS)
        D._SUB_OPCODE_FOR_NAME[name] = row
        shas = {}
        for ver in ("v3", "v4"):
            s = DveOpSpec(name=name, opcode=row, uops=lower(spec, ver=ver),
                          rd1_en=_has_src1(spec))
            shas[ver] = s.sha(ver)
        op = D.DveOp(name, spec, subdim=False, uops_sha=shas)
        D.OPS.append(op)
        D.CUSTOM_DVE_SPECS[name] = spec
        _DVE_OPS[name] = op
        return op

    # gidx is int8-encoded: bucket b -> b-2 in {-2..2}, masked -> 64.
    # Table values arrive pre-shifted by +SHIFT; the exp pass bias subtracts
    # SHIFT, so an unmatched index (masked) nets -SHIFT = a huge negative
    # logit whose exp underflows to 0, with no explicit mask compare.
    # pass A: encoded {0,1,2} = buckets {2,3,4} via s0/s1/in1(C3-spill).
    body_a = (eq(Src0, Zero) * C0 + eq(Src0, One) * C1
              + eq(Src0, TWO) * C3)
    build("MHA_LUT_A", Spec(
        body=_spill_c3_to_src1(body_a),
        reference=lambda in0, in1, s0, s1, imm2: (
            (in0 == 0) * s0 + (in0 == 1) * s1
            + (in0 == 2) * in1).astype(np.float32),
    ))
    # pass B: encoded {-1,-2} = buckets {1,0} added onto pass A's output.
    body_b = Src1 + eq(Src0, NEG1) * C0 + eq(Src0, NEG2) * C1
    build("MHA_LUT_B", Spec(
        body=body_b,
        reference=lambda in0, in1, s0, s1, imm2: (
            in1 + (in0 == -1) * s0 + (in0 == -2) * s1).astype(np.float32),
    ))
    # combine: attn = T^2 * rinv[q] * cinvB
    build("MHA_COMBINE", Spec(
        body=sq(Src0) * C0 * Src1,
        reference=lambda in0, in1, s0, s1, imm2: (
            in0.astype(np.float32) ** 2 * s0 * in1).astype(np.float32),
    ))
    return _DVE_OPS


# --------------------------------------------------------------------------
# The per-core Bass program (SPMD: identical program, per-core input values).
# --------------------------------------------------------------------------
_NC_CACHE = {}


def build_nc(n=N_FULL):
    if n in _NC_CACHE:
        return _NC_CACHE[n]
    ops = _register_dve_ops()
    LUT_A, LUT_B, COMBINE = ops["MHA_LUT_A"], ops["MHA_LUT_B"], ops["MHA_COMBINE"]
    from concourse.masks import make_identity

    nt = n // 128          # q/k tiles of 128
    nb = n // 512          # 512-wide PSUM bank chunks
    ncc = DM // 128        # channel chunks

    nc = bacc.Bacc("TRN2", num_devices=H)

    # ---- I/O ----
    xq_d = nc.dram_tensor("x_q", [n, DM], FP32, kind="ExternalInput")
    xk_d = nc.dram_tensor("x_k", [n, DM], FP32, kind="ExternalInput")
    gidx_d = nc.dram_tensor("gidx", [n, n], I8, kind="ExternalInput")
    wqT_d = nc.dram_tensor("wqT", [DM, DK], FP32, kind="ExternalInput")
    wkT_d = nc.dram_tensor("wkT", [DM, DK], FP32, kind="ExternalInput")
    wvT_d = nc.dram_tensor("wvT", [DM, DK], FP32, kind="ExternalInput")
    wgT_d = nc.dram_tensor("wgT", [DM, DK], FP32, kind="ExternalInput")
    woT_d = nc.dram_tensor("woT", [DM, DM], FP32, kind="ExternalInput")
    relT_d = nc.dram_tensor("rel_kT", [DK, PK], FP32, kind="ExternalInput")
    lnw_d = nc.dram_tensor("ln_w", [DM], FP32, kind="ExternalInput")
    lnb_d = nc.dram_tensor("ln_b", [DM], FP32, kind="ExternalInput")
    bg_d = nc.dram_tensor("bg_h", [DK, 1], FP32, kind="ExternalInput")
    bo_d = nc.dram_tensor("bo", [1, DM], FP32, kind="ExternalInput")

    attn_d = nc.dram_tensor("attn", [n, n], FP32, kind="ExternalOutput")
    out_d = nc.dram_tensor("out", [n, DM], FP32, kind="ExternalOutput")

    cc_in = nc.dram_tensor("cc_in", [DK, n], FP32)
    cc_out = nc.dram_tensor("cc_out", [DM, n], FP32, addr_space="Shared")

    with tile.TileContext(nc) as tc:
        with (
            tc.tile_pool(name="const", bufs=1) as const,
            tc.tile_pool(name="small", bufs=1) as small,
            tc.tile_pool(name="tslab", bufs=1) as tslab,
        ):
            # ---------- constants ----------
            ident = const.tile([128, 128], FP32)
            make_identity(nc, ident)
            identb = const.tile([128, 128], BF16)
            nc.vector.tensor_copy(identb, ident)
            ones_col = const.tile([128, 1], FP32)
            nc.vector.memset(ones_col, 1.0)
            ones_colb = const.tile([128, 128], BF16)
            nc.vector.memset(ones_colb, 1.0)
            ones_row = const.tile([1, 128], FP32)
            nc.vector.memset(ones_row, 1.0)
            ones_colr = const.tile([128, 1], F32R)
            nc.vector.tensor_copy(ones_colr, ones_col)
            identr = const.tile([128, 128], F32R)
            nc.vector.tensor_copy(identr, ident)
            ones_rowb = const.tile([1, 128], BF16)
            nc.vector.tensor_copy(ones_rowb, ones_row)
            eps_t = const.tile([1, 1], FP32)
            nc.vector.memset(eps_t, EPS)
            eps128 = const.tile([128, 1], FP32)
            nc.vector.memset(eps128, EPS)

            relT = const.tile([DK, PK], FP32)
            nc.sync.dma_start(out=relT, in_=relT_d[:, :])
            bg_t = const.tile([DK, 1], FP32)
            nc.sync.dma_start(out=bg_t, in_=bg_d[:, :])
            bo_t = const.tile([1, DM], FP32)
            nc.sync.dma_start(out=bo_t, in_=bo_d[:, :])
            # ln_w / ln_b in per-partition chunk form [128, ncc]
            lnw_pp = const.tile([128, ncc], FP32)
            nc.sync.dma_start(
                out=lnw_pp, in_=lnw_d.rearrange("(cc p) -> p cc", p=128))
            lnb_pp = const.tile([128, ncc], FP32)
            nc.sync.dma_start(
                out=lnb_pp, in_=lnb_d.rearrange("(cc p) -> p cc", p=128))

            wT = {}
            for name, dram in (("q", wqT_d), ("k", wkT_d), ("v", wvT_d),
                               ("g", wgT_d)):
                t = const.tile([128, ncc, DK], FP32, tag=f"w{name}")
                nc.sync.dma_start(
                    out=t, in_=dram.rearrange("(cc p) d -> p cc d", p=128))
                wT[name] = t
            woT = const.tile([128, ncc, DM], FP32)
            nc.sync.dma_start(
                out=woT, in_=woT_d.rearrange("(cc p) d -> p cc d", p=128))

            # ================= phase 0: LN + projections =================
            with (
                tc.tile_pool(name="xload", bufs=3) as xload,
                tc.tile_pool(name="xt", bufs=1) as xtp,
                tc.tile_pool(name="sqbuf", bufs=2) as sqp,
                tc.tile_pool(name="p0tmp", bufs=1) as p0tmp,
                tc.tile_pool(name="ps_row", bufs=1, space="PSUM") as ps_row,
                tc.tile_pool(name="ps0", bufs=2, space="PSUM") as ps0,
            ):
                rows_t = small.tile([9, n], FP32, tag="rows")
                R_MU = {"q": 0, "k": 3}
                R_IRS = {"q": 1, "k": 4}
                R_RSTD = {"q": 2, "k": 5}
                # ---------- load x and transpose to [DM, n] ----------
                xT = {}
                for key, dram in (("q", xq_d), ("k", xk_d)):
                    tiles = [xtp.tile([128, n], F32R, tag=f"xt{key}{cc}",
                                      name=f"xt{key}{cc}")
                             for cc in range(ncc)]
                    xT[key] = tiles
                    for i in range(nt):
                        xt_in = xload.tile([128, DM], FP32, tag="xin")
                        nc.sync.dma_start(
                            out=xt_in, in_=dram[i * 128:(i + 1) * 128, :])
                        ps = ps0.tile([128, 512], FP32, tag="ps0")
                        for cc in range(ncc):
                            nc.tensor.transpose(
                                ps[:, cc * 128:(cc + 1) * 128],
                                xt_in[:, cc * 128:(cc + 1) * 128], ident)
                        for cc in range(ncc):
                            nc.scalar.copy(
                                out=xT[key][cc][:, i * 128:(i + 1) * 128],
                                in_=ps[:, cc * 128:(cc + 1) * 128])

                # ---------- LN stats (mu, 1/rstd, rstd as [1, n] rows) -----
                rows = {}
                for key in ("q", "k"):
                    mu_ps = ps_row.tile([1, n], FP32, tag="row")
                    for cc in range(ncc):
                        for b in range(nb):
                            nc.tensor.matmul(
                                mu_ps[:, b * 512:(b + 1) * 512],
                                lhsT=ones_col.bitcast(F32R),
                                rhs=xT[key][cc][:, b * 512:(b + 1) * 512]
                                .bitcast(F32R),
                                start=(cc == 0), stop=(cc == ncc - 1))
                    mu_row = rows_t[R_MU[key]:R_MU[key] + 1, :]
                    nc.scalar.activation(mu_row, mu_ps, AF.Copy,
                                         scale=1.0 / DM)

                    m2_ps = ps_row.tile([1, n], FP32, tag="row")
                    for cc in range(ncc):
                        xsq = sqp.tile([128, n], FP32, tag="xsq")
                        nc.gpsimd.tensor_mul(xsq, xT[key][cc], xT[key][cc])
                        for b in range(nb):
                            nc.tensor.matmul(
                                m2_ps[:, b * 512:(b + 1) * 512],
                                lhsT=ones_col.bitcast(F32R),
                                rhs=xsq[:, b * 512:(b + 1) * 512]
                                .bitcast(F32R),
                                start=(cc == 0), stop=(cc == ncc - 1))
                    var_row = rows_t[6:7, :]
                    nc.scalar.activation(var_row, m2_ps, AF.Copy,
                                         scale=1.0 / DM)
                    musq = rows_t[7:8, :]
                    nc.vector.tensor_mul(musq, mu_row, mu_row)
                    nc.vector.tensor_sub(var_row, var_row, musq)
                    irs_row = rows_t[R_IRS[key]:R_IRS[key] + 1, :]
                    nc.scalar.activation(irs_row, var_row, AF.Sqrt,
                                         bias=eps_t)
                    rstd_row = rows_t[R_RSTD[key]:R_RSTD[key] + 1, :]
                    nc.vector.reciprocal(rstd_row, irs_row)
                    rows[key] = (mu_row, irs_row, rstd_row)

                # rstd_q per-partition [128, nt] (DMA shuffle), * 0.125
                def row_to_pp(row, dst):
                    for t in range(nt):
                        nc.sync.dma_start(
                            out=dst[:, t:t + 1],
                            in_=row[0:1, t * 128:(t + 1) * 128])

                sc_pp = small.tile([128, nt], FP32, tag="scpp")
                row_to_pp(rows["q"][2], sc_pp)
                nc.vector.tensor_scalar_mul(sc_pp, sc_pp, 0.125)
                rstdk_pp = small.tile([128, nt], FP32, tag="rkpp")
                row_to_pp(rows["k"][2], rstdk_pp)

                # rstd broadcast tiles [DK, n] for the k / gate folds
                def row_bcast(row, tag):
                    out_t = p0tmp.tile([DK, n], FP32, tag=tag, name=tag)
                    for b in range(nb):
                        psb = ps0.tile([128, 512], FP32, tag="ps0")
                        nc.tensor.matmul(
                            psb[:DK, :], lhsT=ones_row[:, :DK],
                            rhs=row[:, b * 512:(b + 1) * 512])
                        nc.scalar.copy(out=out_t[:, b * 512:(b + 1) * 512],
                                       in_=psb[:DK, :])
                    return out_t

                rstd_kB = row_bcast(rows["k"][2], "rkB")
                rstd_qB = row_bcast(rows["q"][2], "rqB")

                # ---------- projections ----------
                def project(key, xkey, apply):
                    """[DK, n] psum = W'^T x^T - wbar (x) mu + beta (x) irs"""
                    wt = wT[key]
                    mu_row, irs_row, _ = rows[xkey]
                    wp = p0tmp.tile([128, ncc, DK], FP32, tag="wp",
                                    name=f"wp{key}")
                    for cc in range(ncc):
                        nc.vector.tensor_scalar_mul(
                            wp[:, cc, :], wt[:, cc, :], lnw_pp[:, cc:cc + 1])
                    wbar_ps = ps0.tile([128, 512], FP32, tag="ps0")
                    beta_ps = ps0.tile([128, 512], FP32, tag="ps0")
                    for cc in range(ncc):
                        nc.tensor.matmul(
                            wbar_ps[:1, :DK], lhsT=ones_col.bitcast(F32R),
                            rhs=wp[:, cc, :].bitcast(F32R),
                            start=(cc == 0), stop=(cc == ncc - 1))
                        nc.tensor.matmul(
                            beta_ps[:1, :DK],
                            lhsT=lnb_pp[:, cc:cc + 1].bitcast(F32R),
                            rhs=wt[:, cc, :].bitcast(F32R),
                            start=(cc == 0), stop=(cc == ncc - 1))
                    nwbar = p0tmp.tile([1, DK], FP32, tag="nwbar",
                                       name=f"nwbar{key}")
                    nc.scalar.activation(nwbar, wbar_ps[:1, :DK], AF.Copy,
                                         scale=-1.0)
                    beta = p0tmp.tile([1, DK], FP32, tag="beta",
                                      name=f"beta{key}")
                    nc.scalar.copy(out=beta, in_=beta_ps[:1, :DK])

                    proj_ps = ps_row.tile([DK, n], FP32, tag="row")
                    for b in range(nb):
                        sl = slice(b * 512, (b + 1) * 512)
                        for cc in range(ncc):
                            nc.tensor.matmul(
                                proj_ps[:, sl],
                                lhsT=wp[:, cc, :].bitcast(F32R),
                                rhs=xT[xkey][cc][:, sl].bitcast(F32R),
                                start=(cc == 0), stop=False)
                        nc.tensor.matmul(
                            proj_ps[:, sl], lhsT=nwbar.bitcast(F32R),
                            rhs=mu_row[:, sl].bitcast(F32R),
                            start=False, stop=False)
                        nc.tensor.matmul(
                            proj_ps[:, sl], lhsT=beta.bitcast(F32R),
                            rhs=irs_row[:, sl].bitcast(F32R),
                            start=False, stop=True)
                    return apply(proj_ps)

                def to_sbuf(tag):
                    def f(ps):
                        pool = small if tag == "q1aT" else p0tmp
                        t = pool.tile([DK, n], FP32, tag=tag, name=tag)
                        nc.scalar.copy(out=t, in_=ps)
                        return t
                    return f

                q1aT = project("q", "q", to_sbuf("q1aT"))

                def k_apply(ps):
                    t = small.tile([DK, n], F32R, tag="k1T")
                    nc.vector.tensor_mul(t, ps, rstd_kB)
                    return t
                k1T = project("k", "k", k_apply)

                v1aT = project("v", "k", to_sbuf("v1aT"))

                def g_apply(ps):
                    t = p0tmp.tile([DK, n], FP32, tag="gpre", name="gpre")
                    nc.vector.tensor_mul(t, ps, rstd_qB)
                    g = p0tmp.tile([DK, n], FP32, tag="gateT", name="gateT")
                    nc.scalar.activation(g, t, AF.Sigmoid, bias=bg_t)
                    return g
                gateT = project("g", "q", g_apply)

                # v1 in k-partition layout [128, nt, DK] bf16, rstd_k applied
                v1 = small.tile([128, nt, DK], BF16, tag="v1")
                for i in range(nt):
                    psv = ps0.tile([128, 512], FP32, tag="ps0")
                    nc.tensor.transpose(
                        psv[:, :DK], v1aT[:, i * 128:(i + 1) * 128],
                        ident[:DK, :DK])
                    nc.vector.tensor_scalar_mul(
                        v1[:, i, :], psv[:, :DK], rstdk_pp[:, i:i + 1])

                # gate in q-partition layout [128, nt, DK] f32
                gate_qp = small.tile([128, nt, DK], FP32, tag="gateqp")
                for i in range(nt):
                    psg = ps0.tile([128, 512], FP32, tag="ps0")
                    nc.tensor.transpose(
                        psg[:, :DK], gateT[:, i * 128:(i + 1) * 128],
                        ident[:DK, :DK])
                    nc.scalar.copy(out=gate_qp[:, i, :], in_=psg[:, :DK])

                # qk_rel coefficients [128, nt, PK] (a-form, pre-rstd)
                qk5 = small.tile([128, nt, PK], FP32, tag="qk5")
                for i in range(nt):
                    ps5 = ps0.tile([128, 512], FP32, tag="ps0")
                    nc.tensor.matmul(
                        ps5[:, :PK],
                        lhsT=q1aT[:, i * 128:(i + 1) * 128].bitcast(FP32),
                        rhs=relT)
                    nc.vector.tensor_scalar_add(qk5[:, i, :], ps5[:, :PK],
                                                SHIFT)

            # ========= phase A: logits -> T (bf16 slab), row/col sums =======
            T_tiles = []
            with (
                tc.tile_pool(name="gidx", bufs=6) as gidxp,
                tc.tile_pool(name="lut", bufs=6) as lutp,
                tc.tile_pool(name="ps_logit", bufs=1, space="PSUM") as ps_logit,
                tc.tile_pool(name="ps_cs", bufs=1, space="PSUM") as ps_cs,
            ):
                rs_all = small.tile([128, nt], FP32, tag="rs")
                cs_ps = ps_cs.tile([128, n], FP32)
                for i in range(nt):
                    g_t = gidxp.tile([128, n], I8, tag="g")
                    nc.sync.dma_start(
                        out=g_t, in_=gidx_d[i * 128:(i + 1) * 128, :])
                    lps = ps_logit.tile([128, n], FP32, tag="lg")
                    for b in range(nb):
                        sl = slice(b * 512, (b + 1) * 512)
                        nc.tensor.matmul(
                            lps[:, sl],
                            lhsT=q1aT[:, i * 128:(i + 1) * 128].bitcast(F32R),
                            rhs=k1T[:, sl].bitcast(F32R),
                            start=True, stop=False)
                    lutA = lutp.tile([128, n], FP32, tag="lut")
                    nc.vector._custom_dve(
                        LUT_A, out=lutA, in0=g_t, in1=qk5[:, i, 4:5],
                        s0=qk5[:, i, 2:3], s1=qk5[:, i, 3:4])
                    lutB = lutp.tile([128, n], FP32, tag="lut")
                    nc.vector._custom_dve(
                        LUT_B, out=lutB, in0=g_t, in1=lutA,
                        s0=qk5[:, i, 1:2], s1=qk5[:, i, 0:1])
                    for b in range(nb):
                        sl = slice(b * 512, (b + 1) * 512)
                        nc.tensor.matmul(
                            lps[:, sl], lhsT=ident.bitcast(F32R),
                            rhs=lutB[:, sl].bitcast(F32R),
                            start=False, stop=True)
                    T_t = tslab.tile([128, n], BF16, tag=f"T{i}")
                    nc.scalar.activation(
                        T_t, lps, AF.Exp, scale=sc_pp[:, i:i + 1],
                        bias=nbias[:, i:i + 1],
                        accum_out=rs_all[:, i:i + 1])
                    T_tiles.append(T_t)
                    for b in range(nb):
                        sl = slice(b * 512, (b + 1) * 512)
                        nc.tensor.matmul(
                            cs_ps[:, sl], lhsT=ones_colb,
                            rhs=T_t[:, sl], start=(i == 0),
                            stop=(i == nt - 1))

                rinv = small.tile([128, nt], FP32, tag="rinv")
                nc.vector.reciprocal(rinv, rs_all)
                cinvB = small.tile([128, n], FP32, tag="cinvB")
                nc.vector.reciprocal(cinvB, cs_ps)

            # sqrt(cinv) in per-partition form [128, nt]
            sqc_row = rows_t[8:9, :]
            nc.scalar.activation(sqc_row, cinvB[:1, :], AF.Sqrt)
            sqc_pp = small.tile([128, nt], FP32, tag="sqcpp")
            for t in range(nt):
                nc.sync.dma_start(
                    out=sqc_pp[:, t:t + 1],
                    in_=sqc_row[0:1, t * 128:(t + 1) * 128])

            # ========= phase B: attn out, attn @ v, gate, Wo ===============
            with (
                tc.tile_pool(name="attnout", bufs=3) as attnp,
                tc.tile_pool(name="t2t", bufs=3) as t2tp,
                tc.tile_pool(name="fin", bufs=3) as finp,
                tc.tile_pool(name="ps_tt", bufs=2, space="PSUM") as ps_tt,
                tc.tile_pool(name="ps_av", bufs=2, space="PSUM") as ps_av,
                tc.tile_pool(name="ps_og", bufs=2, space="PSUM") as ps_og,
            ):
                ogT = small.tile([DK, n], FP32, tag="ogT")
                for i in range(nt):
                    a_t = attnp.tile([128, n], FP32, tag="a")
                    nc.vector._custom_dve(
                        COMBINE, out=a_t, in0=T_tiles[i], in1=cinvB,
                        s0=rinv[:, i:i + 1])
                    nc.sync.dma_start(
                        out=attn_d[i * 128:(i + 1) * 128, :], in_=a_t)
                    # AV: out_g[q, :] = rinv[q] * sum_k (T*sqrt(cinv))^2 v1[k]
                    avp = ps_av.tile([128, DK], FP32, tag="av")
                    for jj in range(nt // 4):
                        tt = ps_tt.tile([128, 512], BF16, tag="tt")
                        for u in range(4):
                            j = jj * 4 + u
                            nc.tensor.transpose(
                                tt[:, u * 128:(u + 1) * 128],
                                T_tiles[i][:, j * 128:(j + 1) * 128], identb)
                        t2 = t2tp.tile([128, 512], BF16, tag="t2")
                        for u in range(4):
                            j = jj * 4 + u
                            nc.scalar.activation(
                                t2[:, u * 128:(u + 1) * 128],
                                tt[:, u * 128:(u + 1) * 128],
                                AF.Square, scale=sqc_pp[:, j:j + 1])
                        for u in range(4):
                            j = jj * 4 + u
                            nc.tensor.matmul(
                                avp, lhsT=t2[:, u * 128:(u + 1) * 128],
                                rhs=v1[:, j, :],
                                start=(j == 0), stop=(j == nt - 1))
                    og = finp.tile([128, DK], FP32, tag="og")
                    nc.vector.scalar_tensor_tensor(
                        out=og, in0=avp, scalar=rinv[:, i:i + 1],
                        in1=gate_qp[:, i, :], op0=ALU.mult, op1=ALU.mult)
                    ogp_ps = ps_og.tile([128, 128], FP32, tag="ogt")
                    nc.tensor.transpose(ogp_ps[:DK, :], og, ident)
                    nc.scalar.copy(
                        out=ogT[:, i * 128:(i + 1) * 128],
                        in_=ogp_ps[:DK, :])

                # ---------- AllGather heads + output projection ----------
                nc.sync.dma_start(out=cc_in[:, :], in_=ogT)
                nc.gpsimd.collective_compute(
                    "AllGather", ALU.bypass,
                    replica_groups=[list(range(H))],
                    ins=[cc_in[:, :]], outs=[cc_out[:, :]])
                GT = []
                for cc in range(ncc):
                    t = small.tile([128, n], FP32, tag=f"gt{cc}")
                    nc.sync.dma_start(
                        out=t, in_=cc_out[cc * 128:(cc + 1) * 128, :])
                    GT.append(t)
                for i in range(nt):
                    ps_o = ps_og.tile([128, DM], FP32, tag="fin")
                    for cc in range(ncc):
                        nc.tensor.matmul(
                            ps_o,
                            lhsT=GT[cc][:, i * 128:(i + 1) * 128]
                            .bitcast(F32R),
                            rhs=woT[:, cc, :].bitcast(F32R),
                            start=(cc == 0), stop=False)
                    nc.tensor.matmul(
                        ps_o, lhsT=ones_row.bitcast(F32R),
                        rhs=bo_t.bitcast(F32R), start=False, stop=True)
                    o_t = finp.tile([128, DM], FP32, tag="fout")
                    nc.scalar.copy(out=o_t, in_=ps_o)
                    nc.sync.dma_start(
                        out=out_d[i * 128:(i + 1) * 128, :], in_=o_t)

    nc.compile()
    _NC_CACHE[n] = nc
    return nc


# --------------------------------------------------------------------------
# Host entry: shard, run SPMD on 8 cores, gather.
# --------------------------------------------------------------------------
def make_in_maps(x_q, x_k, gpm, mask, ln_w, ln_b, Wq, Wk, Wv, Wg, bg, Wo, bo,
                 rel_k):
    f32 = np.float32
    gidx = np.where(np.asarray(mask, bool),
                    np.asarray(gpm) - 2, 64).astype(np.int8)
    base = {
        "x_q": np.ascontiguousarray(np.asarray(x_q, f32)),
        "x_k": np.ascontiguousarray(np.asarray(x_k, f32)),
        "gidx": gidx,
        "woT": np.ascontiguousarray(np.asarray(Wo, f32).T),
        "rel_kT": np.ascontiguousarray(np.asarray(rel_k, f32).T),
        "ln_w": np.ascontiguousarray(np.asarray(ln_w, f32)),
        "ln_b": np.ascontiguousarray(np.asarray(ln_b, f32)),
        "bo": np.ascontiguousarray(np.asarray(bo, f32).reshape(1, DM)),
    }
    in_maps = []
    for h in range(H):
        sl = slice(h * DK, (h + 1) * DK)
        m = dict(base)
        m["wqT"] = np.ascontiguousarray(np.asarray(Wq, f32)[sl].T)
        m["wkT"] = np.ascontiguousarray(np.asarray(Wk, f32)[sl].T)
        m["wvT"] = np.ascontiguousarray(np.asarray(Wv, f32)[sl].T)
        m["wgT"] = np.ascontiguousarray(np.asarray(Wg, f32)[sl].T)
        m["bg_h"] = np.ascontiguousarray(
            np.asarray(bg, f32)[sl].reshape(DK, 1))
        in_maps.append(m)
    return in_maps


def kernel(**inputs):
    from concourse.bass_utils import run_bass_kernel_spmd

    nc = build_nc(N_FULL)
    in_maps = make_in_maps(**inputs)
    res = run_bass_kernel_spmd(nc, in_maps, list(range(H)))
    out = np.asarray(res.results[0]["out"], np.float32)
    attn = np.stack(
        [np.asarray(res.results[h]["attn"], np.float32) for h in range(H)], 0)
    return out, attn
